# revision 1
# baseline (speedup 1.0000x reference)
"""Trainium2 Bass kernel for nn_NodeNetwork (GNN message passing).

Algebraic reformulation: the reference computes
    bo = Ro^T X ; bi = Ri^T X          [E, D]
    mi = (Ri . e) bo ; mo = (Ro . e) bi  [N, D]
which collapses (for ANY Ri/Ro) to
    mi = S X,   mo = S^T X,   S = (Ri . e) Ro^T   in R^{N x N}
S is only [4096, 4096] per batch (vs [4096, 16384] incidence), and since
Ri/Ro are one-hot it is built on the host by a 16K-element scatter-add.
The device kernel is then two dense [N,N]x[N,D] matmuls + the MLP:
    h = tanh([mi, mo, X] @ W1 + b1); y = tanh(h @ W2 + b2)

Sharding: 8 cores = 2 batches x 4 row-slices of N (NSL = 1024 rows each).
Core (b, s) computes y[b, s*NSL:(s+1)*NSL, :] outright -- no collectives.
Per-core HBM traffic: two fp16 [4096, 1024] slices of S / S^T = 16 MB,
streamed as 1 MB linear DMAs (rows are pre-interleaved on the host so each
1 MB chunk maps to [128 partitions x 8 KB contiguous]).
"""

import numpy as np

import concourse.bass as bass
import concourse.mybir as mybir
import concourse.tile as tile
from concourse import bacc
from concourse.bass_utils import run_bass_kernel_spmd

B, N, E, D, OUT = 2, 4096, 16384, 64, 64
NCORES = 8
G = 4                    # cores per batch
NSL = N // G             # 1024 output rows per core
MQ = 2                   # m-loop: 2 DMA chunks of 2048 m-rows (4 MB each)
MR = 16                  # sub-blocks per chunk (2048 = 16 * 128)
NH = NSL // 512          # 2 psum halves of 512 cols

F32 = mybir.dt.float32
F16 = mybir.dt.float16
F8 = mybir.dt.float8e3     # e3m4
F8MAX = 15.5
S_SCALE = 8.0              # S stored as S*8 in fp8
XH_SCALE = 2.0             # X_hi stored as X*2
XL_SCALE = 64.0            # X_lo stored as (X - dehi)*64

_cache = {}


def _build_program(repeat=1, mq=MQ, coltile=True, sbufs=2, ssring=True, mi8=False,
                   pbufs=7, ogp=False, unroll=16, tri=False, out16=False):
    mr = N // (128 * mq)
    nc = bacc.Bacc(
        "TRN2",
        target_bir_lowering=False,
        debug=False,
        num_devices=NCORES,
    )

    # S^T[:, sl] and S[:, sl], rows interleaved: param[p, ((mq*MR)+k)*NSL + j]
    # = A[mq*512 + p*MR + k, j]
    sts = nc.declare_dram_parameter(
        "sts", [128, mq * mr * NSL], F8 if mi8 else F16, isOutput=False
    )
    ss = nc.declare_dram_parameter("ss", [128, mq * mr * NSL], F16, isOutput=False)
    # X fp16, same row interleave: x16i[p, ((mq*MR)+k)*D + d] = X[mq*512+p*MR+k, d]
    x16i = nc.declare_dram_parameter("x16i", [128, mq * mr * D], F16, isOutput=False)
    if mi8:
        x8hi = nc.declare_dram_parameter("x8hi", [128, mq * mr * D], F8, isOutput=False)
        x8lo = nc.declare_dram_parameter("x8lo", [128, mq * mr * D], F8, isOutput=False)
        w1a512 = nc.declare_dram_parameter("w1a512", [OUT, OUT], F32, isOutput=False)
    # X^T f32 for this core's slice (for the W1c fold)
    xtsl = nc.declare_dram_parameter("xtsl", [OUT, NSL], F32, isOutput=False)
    w1ab = nc.declare_dram_parameter("w1ab", [128, OUT], F32, isOutput=False)
    w1c = nc.declare_dram_parameter("w1c", [OUT, OUT], F32, isOutput=False)
    w2 = nc.declare_dram_parameter("w2", [OUT, OUT], F32, isOutput=False)
    b1d = nc.declare_dram_parameter("b1d", [OUT, 1], F32, isOutput=False)
    b2d = nc.declare_dram_parameter("b2d", [OUT, 1], F32, isOutput=False)
    out = nc.declare_dram_parameter(
        "out", [OUT, NSL], F16 if out16 else F32, isOutput=True
    )

    with tile.TileContext(nc) as tc:
        with (
            tc.tile_pool(name="const", bufs=1) as cpool,
            tc.tile_pool(name="stream", bufs=sbufs) as spool,
            tc.tile_pool(name="stage", bufs=4) as stpool,
            tc.tile_pool(name="psum", bufs=pbufs, space="PSUM") as ppool,
        ):
            x16_sb = cpool.tile([128, mq * mr * D], F16)
            nc.sync.dma_start(x16_sb[:], x16i[:])
            if mi8:
                x8hi_sb = cpool.tile([128, mq * mr * D], F8)
                nc.sync.dma_start(x8hi_sb[:], x8hi[:])
                x8lo_sb = cpool.tile([128, mq * mr * D], F8)
                nc.sync.dma_start(x8lo_sb[:], x8lo[:])
                w1a512_sb = cpool.tile([OUT, OUT], F32)
                nc.sync.dma_start(w1a512_sb[:], w1a512[:])
            xtsl_sb = cpool.tile([OUT, NSL], F32)
            nc.sync.dma_start(xtsl_sb[:], xtsl[:])
            w1ab_sb = cpool.tile([128, OUT], F32)
            nc.sync.dma_start(w1ab_sb[:], w1ab[:])
            w1c_sb = cpool.tile([OUT, OUT], F32)
            nc.sync.dma_start(w1c_sb[:], w1c[:])
            w2_sb = cpool.tile([OUT, OUT], F32)
            nc.sync.dma_start(w2_sb[:], w2[:])
            b1_sb = cpool.tile([OUT, 1], F32)
            nc.sync.dma_start(b1_sb[:], b1d[:])
            b2_sb = cpool.tile([OUT, 1], F32)
            nc.sync.dma_start(b2_sb[:], b2d[:])

            def body(_i=None):
                # [mi; mo] stacked in one PSUM bank per half: mi accumulates on
                # PE column-group (0,0) into rows 0-63, mo on (0,64) into rows
                # 64-127 -- the two streams run concurrently on the array.
                if coltile:
                    ps = [
                        ppool.tile([128, 512], F32, tag="ps", name=f"ps{h}")
                        for h in range(NH)
                    ]
                    if mi8:
                        plo = [
                            ppool.tile([64, 512], F32, tag="ps", name=f"plo{h}")
                            for h in range(NH)
                        ]
                else:
                    pmi = [
                        ppool.tile([64, 512], F32, tag="ps", name=f"pmi{h}")
                        for h in range(NH)
                    ]
                    pmo = [
                        ppool.tile([64, 512], F32, tag="ps", name=f"pmo{h}")
                        for h in range(NH)
                    ]
                for q in range(mq):
                    st_t = spool.tile(
                        [128, mr * NSL], F8 if mi8 else F16, tag="st", name="st_t"
                    )
                    st_eng = nc.gpsimd if (tri and q % 2 == 1) else nc.sync
                    st_eng.dma_start(
                        st_t[:], sts[:, q * mr * NSL : (q + 1) * mr * NSL]
                    )
                    ss_t = spool.tile([128, mr * NSL], F16, tag="ss", name="ss_t")
                    (nc.scalar if ssring else nc.sync).dma_start(
                        ss_t[:], ss[:, q * mr * NSL : (q + 1) * mr * NSL]
                    )
                    st_, sp_ = (q == 0), (q == mq - 1)
                    for k in range(mr):
                        bsl = slice((q * mr + k) * D, (q * mr + k + 1) * D)
                        xsl = x16_sb[:, bsl]
                        first = st_ and k == 0
                        last = sp_ and k == mr - 1
                        for h in range(NH):
                            csl = slice(k * NSL + h * 512, k * NSL + (h + 1) * 512)
                            if coltile and mi8:
                                nc.tensor.matmul(
                                    ps[h][:64, :], x8hi_sb[:, bsl], st_t[:, csl],
                                    start=first, stop=last, tile_position=(0, 0),
                                )
                                nc.tensor.matmul(
                                    ps[h][64:, :], xsl, ss_t[:, csl],
                                    start=first, stop=last, tile_position=(0, 64),
                                )
                                nc.tensor.matmul(
                                    plo[h], x8lo_sb[:, bsl], st_t[:, csl],
                                    start=first, stop=last, tile_position=(0, 0),
                                )
                            elif coltile:
                                nc.tensor.matmul(
                                    ps[h][:64, :], xsl, st_t[:, csl],
                                    start=first, stop=last, tile_position=(0, 0),
                                )
                                nc.tensor.matmul(
                                    ps[h][64:, :], xsl, ss_t[:, csl],
                                    start=first, stop=last, tile_position=(0, 64),
                                )
                            else:
                                nc.tensor.matmul(
                                    pmi[h], xsl, st_t[:, csl],
                                    start=first, stop=last,
                                )
                                nc.tensor.matmul(
                                    pmo[h], xsl, ss_t[:, csl],
                                    start=first, stop=last,
                                )
                # MLP on the accumulated [mi; mo]
                for h in range(NH):
                    osl = slice(h * 512, (h + 1) * 512)
                    mm = stpool.tile([128, 512], F32, tag="mm", name="mm")
                    if coltile:
                        nc.vector.tensor_copy(mm[:], ps[h])
                    else:
                        nc.vector.tensor_copy(mm[:64, :], pmi[h])
                        nc.vector.tensor_copy(mm[64:, :], pmo[h])
                    pz = ppool.tile([64, 512], F32, tag="ps", name="pz")
                    nc.tensor.matmul(pz, w1ab_sb[:], mm[:], start=True, stop=False)
                    if mi8:
                        milo = stpool.tile([64, 512], F32, tag="lo", name="milo")
                        nc.vector.tensor_copy(milo[:], plo[h])
                        nc.tensor.matmul(
                            pz, w1a512_sb[:], milo[:], start=False, stop=False
                        )
                    nc.tensor.matmul(
                        pz, w1c_sb[:], xtsl_sb[:, osl], start=False, stop=True
                    )
                    h_sb = stpool.tile([64, 512], F32, tag="h", name="h_sb")
                    nc.scalar.activation(
                        h_sb[:], pz, mybir.ActivationFunctionType.Tanh, bias=b1_sb[:]
                    )
                    py = ppool.tile([64, 512], F32, tag="ps", name="py")
                    nc.tensor.matmul(py, w2_sb[:], h_sb[:], start=True, stop=True)
                    ysb = stpool.tile(
                        [64, 512], F16 if out16 else F32, tag="y", name="ysb"
                    )
                    nc.scalar.activation(
                        ysb[:], py, mybir.ActivationFunctionType.Tanh, bias=b2_sb[:]
                    )
                    (nc.gpsimd if ogp else nc.sync).dma_start(out[:, osl], ysb[:])

            if repeat == 1:
                body()
            else:
                assert repeat % unroll == 0
                with tc.For_i(0, repeat // unroll, 1) as _i:
                    for _ in range(unroll):
                        body(_i)

    nc.compile()
    return nc


def _interleave_rows(a, mq=MQ):
    """[N, C] -> [128, (mq*mr)*C] with param[p, (q*mr+k)*C + j] = a[q*(N/mq) + p*mr + k, j]."""
    mr = N // (128 * mq)
    c = a.shape[1]
    return np.ascontiguousarray(
        a.reshape(mq, 128, mr, c).transpose(1, 0, 2, 3)
    ).reshape(128, mq * mr * c)


def _onehot_idx(R):
    """Recover per-column argmax index of a one-hot [N, E] matrix (exact for 0/1)."""
    ar = np.arange(N, dtype=np.float32)
    return np.rint(ar @ R).astype(np.int64)


def _q8store(v, scale):
    from ml_dtypes import float8_e3m4
    return np.clip(v * scale, -F8MAX, F8MAX).astype(float8_e3m4)


def make_in_maps(X, e, Ri, Ro, W1, b1, W2, b2, mq=MQ, mi8=False):
    X = np.asarray(X, dtype=np.float32)
    e = np.asarray(e, dtype=np.float32)
    W1 = np.asarray(W1, dtype=np.float32)
    b1 = np.asarray(b1, dtype=np.float32)
    W2 = np.asarray(W2, dtype=np.float32)
    b2 = np.asarray(b2, dtype=np.float32)

    w1ab = np.ascontiguousarray(W1[:128])
    if mi8:
        # mi-hi product carries scale XH_SCALE*S_SCALE; mi-lo XL_SCALE*S_SCALE
        w1ab = np.concatenate(
            [W1[:64] / (XH_SCALE * S_SCALE), W1[64:128]], axis=0
        ).astype(np.float32)
    w1a512 = np.ascontiguousarray(W1[:64] / (XL_SCALE * S_SCALE)).astype(np.float32)
    w1c = np.ascontiguousarray(W1[128:])
    w2c = np.ascontiguousarray(W2)
    b1c = np.ascontiguousarray(b1.reshape(OUT, 1))
    b2c = np.ascontiguousarray(b2.reshape(OUT, 1))

    per_batch = []
    ar_e = np.arange(E)
    for b_ in range(B):
        Rib = np.asarray(Ri[b_], dtype=np.float32)
        Rob = np.asarray(Ro[b_], dtype=np.float32)
        ri = _onehot_idx(Rib)
        ro = _onehot_idx(Rob)
        vi = Rib[ri, ar_e]
        vo = Rob[ro, ar_e]
        # exact when each column has at most one nonzero (the one-hot case);
        # fall back to the dense product otherwise
        if (
            ri.min() < 0 or ri.max() >= N or ro.min() < 0 or ro.max() >= N
            or not np.allclose((Rib != 0).sum(axis=0), 1)
            or not np.allclose((Rob != 0).sum(axis=0), 1)
        ):
            S = (Rib * e[b_]) @ Rob.T
        else:
            S = np.zeros((N, N), dtype=np.float32)
            np.add.at(S, (ri, ro), e[b_] * vi * vo)
        ST = np.ascontiguousarray(S.T)
        xb = X[b_]
        x16i = _interleave_rows(xb.astype(np.float16), mq)
        if mi8:
            xhi = _q8store(xb, XH_SCALE)
            dehi = xhi.astype(np.float32) / XH_SCALE
            xlo = _q8store(xb - dehi, XL_SCALE)
            x8hi_i = _interleave_rows(xhi, mq)
            x8lo_i = _interleave_rows(xlo, mq)
        else:
            x8hi_i = x8lo_i = None
        per_batch.append((S, ST, xb, x16i, x8hi_i, x8lo_i))

    in_maps = []
    for c in range(NCORES):
        b_, s = divmod(c, G)
        S, ST, xb, x16i, x8hi_i, x8lo_i = per_batch[b_]
        sl = slice(s * NSL, (s + 1) * NSL)
        # mi = S X  -> rhs[m, j] = S^T[m, sl], interleaved
        if mi8:
            sts_c = _interleave_rows(_q8store(ST[:, sl], S_SCALE), mq)
        else:
            sts_c = _interleave_rows(ST[:, sl].astype(np.float16), mq)
        # mo = S^T X -> rhs[m, j] = S[m, sl], interleaved
        ss_c = _interleave_rows(S[:, sl].astype(np.float16), mq)
        xtsl = np.ascontiguousarray(xb[sl].T)
        d = {
            "sts": sts_c, "ss": ss_c, "x16i": x16i, "xtsl": xtsl,
            "w1ab": w1ab, "w1c": w1c, "w2": w2c,
            "b1d": b1c, "b2d": b2c,
        }
        if mi8:
            d["x8hi"] = x8hi_i
            d["x8lo"] = x8lo_i
            d["w1a512"] = w1a512
        in_maps.append(d)
    return in_maps


def assemble_output(results):
    y = np.empty((B, N, OUT), dtype=np.float32)
    for c in range(NCORES):
        b_, s = divmod(c, G)
        y[b_, s * NSL : (s + 1) * NSL, :] = results[c]["out"].T
    return y


def get_program(repeat=1, mq=MQ, coltile=True, sbufs=2, ssring=True, mi8=False,
                pbufs=7, ogp=False, unroll=16, tri=False, out16=False):
    key = ("nc", repeat, mq, coltile, sbufs, ssring, mi8, pbufs, ogp, unroll, tri,
           out16)
    if key not in _cache:
        _cache[key] = _build_program(
            repeat, mq=mq, coltile=coltile, sbufs=sbufs, ssring=ssring, mi8=mi8,
            pbufs=pbufs, ogp=ogp, unroll=unroll, tri=tri, out16=out16,
        )
    return _cache[key]


def kernel(X, e, Ri, Ro, W1, b1, W2, b2):
    nc = get_program()
    in_maps = make_in_maps(X, e, Ri, Ro, W1, b1, W2, b2)
    res = run_bass_kernel_spmd(nc, in_maps, list(range(NCORES)))
    return assemble_output(res.results)



# revision 2
# speedup vs baseline: 4.0341x; 4.0341x over previous
"""Trainium2 Bass kernel for nn_NodeNetwork (GNN message passing).

Algebraic reformulation: the reference collapses (for one-hot Ri/Ro) to
    mi = S X,   mo = S^T X,   S = (Ri . e) Ro^T   in R^{N x N}
with S built by a 16K-element scatter-add on the host.  S has only ~E=16K
nonzeros, so instead of streaming dense [N, N] slices (16 MB fp16 per core)
the host COMPACTS the sparse product into per-block gathered operands:

For a block of C=64 output nodes, the <=K_PAD edges targeting those nodes
give a gathered source matrix Xg [K_PAD, D] (rows of X, pure host-side
indexing) and a compacted scatter matrix Sg [K_PAD, C] (one e-value per
edge row, in that edge's target column).  Then
    (mi^T)[:, block] = Xg^T @ Sg
exactly, as KT=3 accumulating [128]x[128,C] matmuls.  Same for mo with
ri/ro swapped.  Per-core traffic drops 16.8 MB -> ~3.4 MB (fp16
throughout, so no fp8 accuracy risk), and the PE does ~16x fewer MACs.

Sharding: 8 cores = 2 batches x 4 row-slices of N (NSL = 1024 rows each).
Core (b, s) computes y[b, s*NSL:(s+1)*NSL, :] outright -- no collectives.
Block fill is Poisson(256); K_PAD=384 is an 8-sigma bound (verified 301
max on the reference inputs); overflow raises (correctness never silent).
"""

import numpy as np

import concourse.bass as bass
import concourse.mybir as mybir
import concourse.tile as tile
from concourse import bacc
from concourse.bass_utils import run_bass_kernel_spmd

B, N, E, D, OUT = 2, 4096, 16384, 64, 64
NCORES = 8
G = 4                    # cores per batch
NSL = N // G             # 1024 output rows per core
C = 64                   # output-node columns per block
KT = 3                   # 128-row k-tiles per block (K_PAD = 384)
K_PAD = KT * 128
NBLK = NSL // C          # 16 blocks per core
NH = NSL // 512          # 2 psum halves of 512 cols
BH = NBLK // NH          # 8 blocks per half

F32 = mybir.dt.float32
F16 = mybir.dt.float16

_cache = {}


def _build_program(repeat=1, unroll=16):
    nc = bacc.Bacc(
        "TRN2",
        target_bir_lowering=False,
        debug=False,
        num_devices=NCORES,
    )

    # Compacted operands, column index (bk*KT + t)*C + j:
    #   x*[p, (bk*KT+t)*C + d] = X[m_edge, d]   (gathered source rows)
    #   s*[p, (bk*KT+t)*C + j] = e_edge         (target col j within block)
    # where edge slot = t*128 + p within block bk.
    xmi = nc.declare_dram_parameter("xmi", [128, NBLK * KT * C], F16, isOutput=False)
    smi = nc.declare_dram_parameter("smi", [128, NBLK * KT * C], F16, isOutput=False)
    xmo = nc.declare_dram_parameter("xmo", [128, NBLK * KT * C], F16, isOutput=False)
    smo = nc.declare_dram_parameter("smo", [128, NBLK * KT * C], F16, isOutput=False)
    # X^T fp16 for this core's slice (for the W1c fold)
    xt16 = nc.declare_dram_parameter("xt16", [OUT, NSL], F16, isOutput=False)
    w1ab = nc.declare_dram_parameter("w1ab", [128, OUT], F16, isOutput=False)
    w1c = nc.declare_dram_parameter("w1c", [OUT, OUT], F16, isOutput=False)
    w2 = nc.declare_dram_parameter("w2", [OUT, OUT], F16, isOutput=False)
    b1d = nc.declare_dram_parameter("b1d", [OUT, 1], F32, isOutput=False)
    b2d = nc.declare_dram_parameter("b2d", [OUT, 1], F32, isOutput=False)
    out = nc.declare_dram_parameter("out", [OUT, NSL], F16, isOutput=True)

    HW = BH * KT * C     # stream columns per half

    with tile.TileContext(nc) as tc:
        with (
            tc.tile_pool(name="const", bufs=1) as cpool,
            tc.tile_pool(name="stream", bufs=2) as spool,
            tc.tile_pool(name="stage", bufs=4) as stpool,
            tc.tile_pool(name="psum", bufs=7, space="PSUM") as ppool,
        ):
            xt_sb = cpool.tile([OUT, NSL], F16)
            nc.sync.dma_start(xt_sb[:], xt16[:])
            w1ab_sb = cpool.tile([128, OUT], F16)
            nc.sync.dma_start(w1ab_sb[:], w1ab[:])
            w1c_sb = cpool.tile([OUT, OUT], F16)
            nc.sync.dma_start(w1c_sb[:], w1c[:])
            w2_sb = cpool.tile([OUT, OUT], F16)
            nc.sync.dma_start(w2_sb[:], w2[:])
            b1_sb = cpool.tile([OUT, 1], F32)
            nc.sync.dma_start(b1_sb[:], b1d[:])
            b2_sb = cpool.tile([OUT, 1], F32)
            nc.sync.dma_start(b2_sb[:], b2d[:])

            def body(_i=None):
                for h in range(NH):
                    hsl = slice(h * HW, (h + 1) * HW)
                    xmi_t = spool.tile([128, HW], F16, tag="xmi", name="xmi_t")
                    nc.sync.dma_start(xmi_t[:], xmi[:, hsl])
                    smi_t = spool.tile([128, HW], F16, tag="smi", name="smi_t")
                    nc.scalar.dma_start(smi_t[:], smi[:, hsl])
                    xmo_t = spool.tile([128, HW], F16, tag="xmo", name="xmo_t")
                    nc.gpsimd.dma_start(xmo_t[:], xmo[:, hsl])
                    smo_t = spool.tile([128, HW], F16, tag="smo", name="smo_t")
                    nc.scalar.dma_start(smo_t[:], smo[:, hsl])

                    # [mi; mo] stacked: mi on PE column-group (0,0) into rows
                    # 0-63, mo on (0,64) into rows 64-127 -- concurrent streams.
                    ps = ppool.tile([128, 512], F32, tag="ps", name="ps")
                    for bk in range(BH):
                        osl = slice(bk * C, (bk + 1) * C)
                        for t in range(KT):
                            csl = slice((bk * KT + t) * C, (bk * KT + t + 1) * C)
                            nc.tensor.matmul(
                                ps[:64, osl], xmi_t[:, csl], smi_t[:, csl],
                                start=(t == 0), stop=(t == KT - 1),
                                tile_position=(0, 0),
                            )
                            nc.tensor.matmul(
                                ps[64:, osl], xmo_t[:, csl], smo_t[:, csl],
                                start=(t == 0), stop=(t == KT - 1),
                                tile_position=(0, 64),
                            )
                    # MLP on the accumulated [mi; mo]
                    osl = slice(h * 512, (h + 1) * 512)
                    mm = stpool.tile([128, 512], F16, tag="mm", name="mm")
                    nc.vector.tensor_copy(mm[:], ps)
                    pz = ppool.tile([64, 512], F32, tag="ps", name="pz")
                    nc.tensor.matmul(pz, w1ab_sb[:], mm[:], start=True, stop=False)
                    nc.tensor.matmul(
                        pz, w1c_sb[:], xt_sb[:, osl], start=False, stop=True
                    )
                    h_sb = stpool.tile([64, 512], F16, tag="h", name="h_sb")
                    nc.scalar.activation(
                        h_sb[:], pz, mybir.ActivationFunctionType.Tanh, bias=b1_sb[:]
                    )
                    py = ppool.tile([64, 512], F32, tag="ps", name="py")
                    nc.tensor.matmul(py, w2_sb[:], h_sb[:], start=True, stop=True)
                    ysb = stpool.tile([64, 512], F16, tag="y", name="ysb")
                    nc.scalar.activation(
                        ysb[:], py, mybir.ActivationFunctionType.Tanh, bias=b2_sb[:]
                    )
                    nc.sync.dma_start(out[:, osl], ysb[:])

            if repeat == 1:
                body()
            else:
                assert repeat % unroll == 0
                with tc.For_i(0, repeat // unroll, 1) as _i:
                    for _ in range(unroll):
                        body(_i)

    nc.compile()
    return nc


def _onehot_idx(R):
    """Recover per-column argmax index of a one-hot [N, E] matrix (exact for 0/1)."""
    ar = np.arange(N, dtype=np.float32)
    return np.rint(ar @ R).astype(np.int64)


def _build_pair(cols, m, v, X16):
    """Compact edges (target col in 0..NSL, source row m, value v) into the
    [128, NBLK*KT*C] gathered-X / scatter-value operand pair."""
    bk = cols >> 6
    order = np.argsort(bk, kind="stable")
    bk_s, j_s, m_s, v_s = bk[order], (cols & 63)[order], m[order], v[order]
    counts = np.bincount(bk_s, minlength=NBLK)
    if counts.max() > K_PAD:
        raise ValueError(
            f"block overflow: {counts.max()} edges in one {C}-node block "
            f"exceeds K_PAD={K_PAD}; recompile with larger KT"
        )
    starts = np.concatenate([[0], np.cumsum(counts)[:-1]])
    pos = np.arange(len(bk_s)) - starts[bk_s]
    t_s = pos >> 7
    p_s = pos & 127
    colbase = (bk_s * KT + t_s) * C
    xg = np.zeros((128, NBLK * KT * C), np.float16)
    sg = np.zeros((128, NBLK * KT * C), np.float16)
    xg[p_s[:, None], colbase[:, None] + np.arange(C)[None, :]] = X16[m_s]
    sg[p_s, colbase + j_s] = v_s
    return xg, sg


def make_in_maps(X, e, Ri, Ro, W1, b1, W2, b2):
    X = np.asarray(X, dtype=np.float32)
    e = np.asarray(e, dtype=np.float32)
    W1 = np.asarray(W1, dtype=np.float32)
    b1 = np.asarray(b1, dtype=np.float32)
    W2 = np.asarray(W2, dtype=np.float32)
    b2 = np.asarray(b2, dtype=np.float32)

    w1ab = np.ascontiguousarray(W1[:128]).astype(np.float16)
    w1c = np.ascontiguousarray(W1[128:]).astype(np.float16)
    w2c = np.ascontiguousarray(W2).astype(np.float16)
    b1c = np.ascontiguousarray(b1.reshape(OUT, 1))
    b2c = np.ascontiguousarray(b2.reshape(OUT, 1))

    per_batch = []
    for b_ in range(B):
        ri = _onehot_idx(np.asarray(Ri[b_], dtype=np.float32))
        ro = _onehot_idx(np.asarray(Ro[b_], dtype=np.float32))
        per_batch.append((ri, ro, e[b_], X[b_], X[b_].astype(np.float16)))

    in_maps = []
    for c in range(NCORES):
        b_, s = divmod(c, G)
        ri, ro, eb, xb, x16 = per_batch[b_]
        lo, hi = s * NSL, (s + 1) * NSL
        # mi[n] = sum_{edges: ri=n} e * X[ro]  -> group by ri, gather X[ro]
        sel = (ri >= lo) & (ri < hi)
        xmi_c, smi_c = _build_pair(ri[sel] - lo, ro[sel], eb[sel], x16)
        # mo[n] = sum_{edges: ro=n} e * X[ri]  -> group by ro, gather X[ri]
        sel = (ro >= lo) & (ro < hi)
        xmo_c, smo_c = _build_pair(ro[sel] - lo, ri[sel], eb[sel], x16)
        in_maps.append({
            "xmi": xmi_c, "smi": smi_c, "xmo": xmo_c, "smo": smo_c,
            "xt16": np.ascontiguousarray(xb[lo:hi].T.astype(np.float16)),
            "w1ab": w1ab, "w1c": w1c, "w2": w2c,
            "b1d": b1c, "b2d": b2c,
        })
    return in_maps


def assemble_output(results):
    y = np.empty((B, N, OUT), dtype=np.float32)
    for c in range(NCORES):
        b_, s = divmod(c, G)
        y[b_, s * NSL : (s + 1) * NSL, :] = results[c]["out"].T
    return y


def get_program(repeat=1, unroll=16):
    key = ("nc", repeat, unroll)
    if key not in _cache:
        _cache[key] = _build_program(repeat, unroll=unroll)
    return _cache[key]


def kernel(X, e, Ri, Ro, W1, b1, W2, b2):
    nc = get_program()
    in_maps = make_in_maps(X, e, Ri, Ro, W1, b1, W2, b2)
    res = run_bass_kernel_spmd(nc, in_maps, list(range(NCORES)))
    return assemble_output(res.results)
